# revision 30
# baseline (speedup 1.0000x reference)
"""Trainium2 Bass kernel for LocalSelfAttention (sliding-window, causal).

Problem: val (S=4096, B=2, D=768); q/k/v projections then Longformer-style
banded causal attention, window = 256 lookback (keys j in [i-256, i]).

Sharding: 8 cores = batch (2) x sequence quarters (4). Each core handles
1024 queries of one batch element and receives a 256-row key/value halo
(recomputed locally from val rows; no inter-core communication).

Math simplifications (exact up to float rounding):
  - bk dropped: per-query additive constant q.bk cancels in softmax.
  - bv added on host at the end: sum_j p_j (v0_j + bv) = (PV)/Z + bv.
  - no max-subtraction in softmax: scores ~ N(0,1), |s| < ~8, exp is safe.
  - 1/sqrt(hd) folded into Wq/bq on host.

v3 layout (all bf16 on the PE):
  - phase A: all q projections (DMA-paced ramp; ACT does the bias adds).
  - m-loop: k-proj(m) chunks, scores for head pair j=m, and v-proj chunks
    interleaved at ~1us granularity so PE / ACT(exp) / DVE(copies+masks)
    all stay fed.
  - scores per (head, kvt 128-key tile): one matmul keys x up-to-384
    contiguous queries (the exact span attending that key tile). Mid
    psum tiles hold a kvt pair at col 0 / 512 (bank-aligned); one wide
    exp covers both blocks. Banded validity = triangle-strip multiplies
    on DVE; per-core combined mask handles the first two (halo) tiles.
  - PV: probs-stationary matmuls accumulate all 12 heads into one
    bank-aligned psum tile per 128-query tile; ones-column in v gives
    softmax denominators; division on host from bf16 outputs.
"""

import os
import numpy as np
import ml_dtypes

S, B, D = 4096, 2, 768
H, HD = 12, 64
W = 256
NCORES = 8
SQ = S // 4            # 1024 queries per core
SKV = SQ + W           # 1280 kv rows (halo)
NQT = SQ // 128        # 8 query tiles
ND = D // 128          # 6 feature tiles (= head pairs)
NKVT = SKV // 128      # 10 kv tiles
VA = HD + 2            # 66: per-head v width incl. ones column + pad
SCALE = 1.0 / np.sqrt(HD).astype(np.float32)  # 0.125

_CACHE = {}


def _qlo(kvt):
    return max(0, (kvt - 2) * 128)


def _qhi(kvt):
    return min(SQ, (kvt + 1) * 128)


def _masks_np(boundary: bool) -> np.ndarray:
    """(128, 896) bf16: [M0 (384) | A (128) | B (128) | A (128) | pad (128)].

    Partition p = key-within-tile, free c = query column within block.
    A[p,c] = 1 iff c >= p   (left strip of interior blocks)
    B[p,c] = 1 iff c <= p   (right strip: col c-256 valid iff c-256 <= p)
    M0 covers the t2=0 tile [kvt0 block (128) | kvt1 block (256)]:
      interior: [B | ones | B]; boundary (seq start): zeros (halo keys < 0).
    [A|B] at 384 serves the interior strip pairs (stride-128 view); the
    [A.. A] pair at 384/640 serves t2=4 via a stride-256 view.
    """
    p = np.arange(128)[:, None]
    c = np.arange(128)[None, :]
    A = (c >= p).astype(np.float32)
    Bm = (c <= p).astype(np.float32)
    ones = np.ones((128, 128), np.float32)
    zero = np.zeros((128, 128), np.float32)
    if boundary:
        M0 = np.zeros((128, 384), np.float32)
    else:
        M0 = np.concatenate([Bm, ones, Bm], axis=1)
    m = np.concatenate([M0, A, Bm, A, zero], axis=1)
    return np.ascontiguousarray(m.astype(ml_dtypes.bfloat16))


# scores psum/probs tile layout per t2 (pair of kv tiles):
# t2=0: kvt0 at [0:128), kvt1 at [128:384), width 512 (psum) / 384 (probs)
# t2=1..3: kvt=2t2 at [0:384), kvt=2t2+1 at [512:896); psum width 1024
#          (bank aligned), probs width 896; [384:512) is junk (one wide exp)
# t2=4: kvt8 at [0:256), kvt9 at [256:384), width 512 / 384
def _ps_width(t2):
    return 512 if t2 in (0, 4) else 1024


def _pr_width(t2):
    return 512 if t2 in (0, 4) else 1024


def _blk_off(kvt):
    t2, kk = divmod(kvt, 2)
    if t2 == 0:
        return 0 if kk == 0 else 128
    if t2 == 4:
        return 0 if kk == 0 else 256
    return 0 if kk == 0 else 512


# PV psum col offset per head (66 fp32 cols; no matmul crosses a 2KB bank)
def _pv_off(h):
    return h * 66 if h < 7 else 512 + (h - 7) * 66


def _build_nc():
    import concourse.bacc as bacc
    import concourse.mybir as mybir
    from concourse.tile import TileContext

    f32 = mybir.dt.float32
    bf16 = mybir.dt.bfloat16
    AF = mybir.ActivationFunctionType

    nc = bacc.Bacc(trn_type="TRN2", debug=False, num_devices=NCORES)

    valT_d = nc.dram_tensor("valT", [D, SKV], bf16, kind="ExternalInput").ap()
    wq_d = nc.dram_tensor("wq", [D, D], bf16, kind="ExternalInput").ap()
    wk_d = nc.dram_tensor("wk", [D, D], bf16, kind="ExternalInput").ap()
    wv_d = nc.dram_tensor("wv", [D, D], bf16, kind="ExternalInput").ap()
    bq_d = nc.dram_tensor("bq", [128, ND], f32, kind="ExternalInput").ap()
    masks_d = nc.dram_tensor("masks", [128, 896], bf16, kind="ExternalInput").ap()
    out_d = nc.dram_tensor("out", [SQ, H * VA], bf16, kind="ExternalOutput").ap()

    with TileContext(nc) as tc:
        with tc.tile_pool(name="persist", bufs=1) as pp:
            qT = [pp.tile([128, SQ], bf16, name=f"qT{m}", tag=f"qT{m}") for m in range(ND)]
            kT = [pp.tile([128, SKV], bf16, name=f"kT{m}", tag=f"kT{m}") for m in range(ND)]
            vaug = [pp.tile([128, H * VA], bf16, name=f"vaug{t}", tag=f"vaug{t}") for t in range(NKVT)]
            bqt = pp.tile([128, ND], f32, name="bqt", tag="bqt")
            maskt = pp.tile([128, 896], bf16, name="maskt", tag="maskt")
            probs = [[pp.tile([128, _pr_width(t2)], bf16, name=f"pr{h}_{t2}", tag=f"pr{h}_{t2}")
                      for t2 in range(5)] for h in range(H)]

            valT_t, wq_t, wk_t, wv_t = [], [], [], []
            with tc.tile_pool(name="weights", bufs=1) as wp:
                # --- input DMAs. Per-queue transfers run strictly in issue
                # order, so priority is enforced within a queue: sync carries
                # valT (needed first) then wk then masks then wv; scalar
                # carries bq+wq (light, frees scalar for ACT work quickly).
                # per-queue transfers run strictly in issue order: sync
                # carries valT (needed first) then wk/masks/wv; scalar carries
                # bq+wq (light; scalar engine is free of DMA work afterwards)
                for k in range(ND):
                    tw = wp.tile([128, D], bf16, name=f"wqt{k}", tag=f"wqt{k}")
                    nc.scalar.dma_start(tw[:], wq_d[k * 128:(k + 1) * 128, :])
                    wq_t.append(tw)
                nc.scalar.dma_start(bqt[:], bq_d)
                # phase A reads only valT cols [W:SKV); send those first for
                # all k-tiles, then the halo cols [0:W) (needed in the m-loop)
                for k in range(ND):
                    tv = wp.tile([128, SKV], bf16, name=f"valTt{k}", tag=f"valTt{k}")
                    nc.sync.dma_start(tv[:, W:SKV], valT_d[k * 128:(k + 1) * 128, W:SKV])
                    valT_t.append(tv)
                for k in range(ND):
                    nc.sync.dma_start(valT_t[k][:, 0:W], valT_d[k * 128:(k + 1) * 128, 0:W])
                for k in range(ND):
                    t = wp.tile([128, D], bf16, name=f"wkt{k}", tag=f"wkt{k}")
                    nc.sync.dma_start(t[:], wk_d[k * 128:(k + 1) * 128, :])
                    wk_t.append(t)
                nc.sync.dma_start(maskt[:], masks_d)
                for k in range(ND):
                    t = wp.tile([128, D], bf16, name=f"wvt{k}", tag=f"wvt{k}")
                    nc.sync.dma_start(t[:], wv_d[k * 128:(k + 1) * 128, :])
                    wv_t.append(t)

                for t in range(NKVT):
                    va3 = vaug[t][:].rearrange("p (h c) -> p h c", c=VA)
                    nc.vector.memset(va3[:, :, HD:HD + 1], 1.0)
                    nc.vector.memset(va3[:, :, HD + 1:VA], 0.0)

                vchunks = [(t, lo, hi, h0) for t in range(NKVT)
                           for lo, hi, h0 in ((0, 512, 0), (512, 768, 8))]
                vidx = 0

                # ---- phase A: all q projections. Six accumulation chains
                # interleaved in 3-matmul half-chains so all chains advance as
                # valT tiles land (DMA-paced ramp), without per-MM bank cycling.
                with tc.tile_pool(name="phA", bufs=1, space="PSUM") as phA:
                    for grp in range(2):
                        chains = [(m, ch) for m in range(3 * grp, 3 * grp + 3)
                                  for ch in range(2)]
                        tiles = [phA.tile([128, 512], f32, name=f"psq{i}", tag=f"psq{i}")
                                 for i in range(6)]
                        for half in range(2):
                            for i, (m, ch) in enumerate(chains):
                                for k in range(3 * half, 3 * half + 3):
                                    nc.tensor.matmul(
                                        tiles[i][:],
                                        wq_t[k][:, m * 128:(m + 1) * 128],
                                        valT_t[k][:, W + ch * 512:W + (ch + 1) * 512],
                                        start=(k == 0), stop=(k == ND - 1))
                        for i, (m, ch) in enumerate(chains):
                            nc.vector.tensor_scalar_add(
                                qT[m][:, ch * 512:(ch + 1) * 512], tiles[i][:],
                                bqt[:, m:m + 1])

                with tc.tile_pool(name="pps", bufs=2, space="PSUM") as pps, \
                     tc.tile_pool(name="scps", bufs=3, space="PSUM") as scp:

                    def v_chunk():
                        nonlocal vidx
                        if vidx >= len(vchunks):
                            return
                        t, lo, hi, h0 = vchunks[vidx]
                        vidx += 1
                        ps = pps.tile([128, hi - lo], f32, name="psv", tag="psproj")
                        for k in range(ND):
                            nc.tensor.matmul(
                                ps[:],
                                valT_t[k][:, t * 128:(t + 1) * 128],
                                wv_t[k][:, lo:hi],
                                start=(k == 0), stop=(k == ND - 1))
                        nh = (hi - lo) // HD
                        src = ps[:].rearrange("p (h c) -> p h c", c=HD)
                        dst = vaug[t][:].rearrange(
                            "p (h c) -> p h c", c=VA)[:, h0:h0 + nh, 0:HD]
                        nc.vector.tensor_copy(dst, src)

                    def scores_t2(m, t2):
                        ps_s = [scp.tile([128, _ps_width(t2)], f32,
                                         name=f"pss{hh}", tag="scores")
                                for hh in range(2)]
                        for kk in range(2):
                            kvt = 2 * t2 + kk
                            qlo, qhi = _qlo(kvt), _qhi(kvt)
                            for hh in range(2):
                                nc.tensor.matmul(
                                    ps_s[hh][:, _blk_off(kvt):_blk_off(kvt) + qhi - qlo],
                                    kT[m][hh * 64:hh * 64 + 64, kvt * 128:(kvt + 1) * 128],
                                    qT[m][hh * 64:hh * 64 + 64, qlo:qhi],
                                    start=True, stop=True,
                                    tile_position=(hh * 64, 0))
                        for hh in range(2):
                            h = 2 * m + hh
                            P = probs[h][t2]
                            if t2 in (0, 4):
                                nc.scalar.activation(P[:, 0:384], ps_s[hh][:, 0:384], AF.Exp)
                            else:
                                nc.scalar.activation(P[:, 0:896], ps_s[hh][:, 0:896], AF.Exp)
                            # banded validity masks (strip pairs as one
                            # strided op: cols {off, off+256}, 128 wide)
                            if t2 == 0:
                                nc.vector.tensor_mul(
                                    P[:, 0:384], P[:, 0:384], maskt[:, 0:384])
                            elif t2 == 4:
                                Pv = P[:, 0:512].rearrange(
                                    "p (s c) -> p s c", c=256)[:, :, 0:128]
                                Mv = maskt[:, 384:896].rearrange(
                                    "p (s c) -> p s c", c=256)[:, :, 0:128]
                                nc.vector.tensor_mul(Pv, Pv, Mv)
                            else:
                                Mv = maskt[:, 384:640].rearrange(
                                    "p (s c) -> p s c", c=128)
                                for off, eng in ((0, nc.gpsimd), (512, nc.vector)):
                                    Pv = P[:, off:off + 512].rearrange(
                                        "p (s c) -> p s c", c=256)[:, :, 0:128]
                                    eng.tensor_mul(Pv, Pv, Mv)

                    # ---- m-loop: k-proj(m) + scores(j=m) + v chunks ----
                    for m in range(ND):
                        nv = (3, 4, 4, 4, 4, 1)[m]
                        for lo, hi in ((0, 512), (512, 1024), (1024, 1280)):
                            ps = pps.tile([128, hi - lo], f32, name="psk", tag="psproj")
                            for k in range(ND):
                                nc.tensor.matmul(
                                    ps[:],
                                    wk_t[k][:, m * 128:(m + 1) * 128],
                                    valT_t[k][:, lo:hi],
                                    start=(k == 0), stop=(k == ND - 1))
                            nc.vector.tensor_copy(kT[m][:, lo:hi], ps[:])
                            if lo == 0:
                                for _ in range((nv + 2) // 3):
                                    v_chunk()
                                scores_t2(m, 0)
                                scores_t2(m, 1)
                            elif lo == 512:
                                for _ in range((nv + 1) // 3):
                                    v_chunk()
                                scores_t2(m, 2)
                                scores_t2(m, 3)
                            else:
                                for _ in range(nv // 3):
                                    v_chunk()
                                scores_t2(m, 4)
                    while vidx < len(vchunks):
                        v_chunk()

                # ---- PV phase ----
                with tc.tile_pool(name="pvps", bufs=3, space="PSUM") as pvp, \
                     tc.tile_pool(name="outp", bufs=2) as op:
                    for qt in range(NQT):
                        ps_o = pvp.tile([128, 1024], f32, name="pso", tag="pv")
                        for h in range(H):
                            for n, kvt in enumerate((qt, qt + 1, qt + 2)):
                                t2 = kvt // 2
                                coff = _blk_off(kvt) + qt * 128 - _qlo(kvt)
                                nc.tensor.matmul(
                                    ps_o[:, _pv_off(h):_pv_off(h) + VA],
                                    probs[h][t2][:, coff:coff + 128],
                                    vaug[kvt][:, h * VA:(h + 1) * VA],
                                    start=(n == 0), stop=(n == 2))
                        osb = op.tile([128, H * VA], bf16, name="osb", tag="osb")
                        nc.vector.tensor_copy(osb[:, 0:462], ps_o[:, 0:462])
                        nc.vector.tensor_copy(osb[:, 462:792], ps_o[:, 512:842])
                        nc.sync.dma_start(out_d[qt * 128:(qt + 1) * 128, :], osb[:])
    nc.compile()
    return nc


def _get_nc():
    if "nc" not in _CACHE:
        _CACHE["nc"] = _build_nc()
    return _CACHE["nc"]


def _install_ntff_hook():
    """Provide antenv.axon_hooks (absent in this image) so bass_utils can
    NTFF-profile under axon, using trn_agent_boot's ctypes hook builder."""
    import sys
    import types
    try:
        from antenv.axon_hooks import get_axon_ntff_profile_hook  # noqa: F401
        return
    except ImportError:
        pass
    try:
        import antenv
        from trn_agent_boot.trn_boot import _ntff_profile_via_ctypes
        hook = _ntff_profile_via_ctypes("/opt/axon/libaxon_pjrt.so")
        mod = types.ModuleType("antenv.axon_hooks")
        mod.get_axon_ntff_profile_hook = lambda: hook
        mod.set_axon_ntff_profile_hook = lambda h: None
        sys.modules["antenv.axon_hooks"] = mod
        antenv.axon_hooks = mod
    except Exception as e:  # profiling is best-effort
        print(f"ntff hook install failed: {e}")


def kernel(val, Wq, bq, Wk, bk, Wv, bv):
    from concourse.bass_utils import run_bass_kernel_spmd

    val = np.asarray(val, dtype=np.float32)
    Wq = np.asarray(Wq, dtype=np.float32)
    bq = np.asarray(bq, dtype=np.float32)
    Wk = np.asarray(Wk, dtype=np.float32)
    Wv = np.asarray(Wv, dtype=np.float32)
    bv = np.asarray(bv, dtype=np.float32)

    bf = ml_dtypes.bfloat16
    wq_s = np.ascontiguousarray((Wq * SCALE).astype(bf))
    bq_s = np.ascontiguousarray((bq * SCALE).reshape(ND, 128).T)
    wk_c = np.ascontiguousarray(Wk.astype(bf))
    wv_c = np.ascontiguousarray(Wv.astype(bf))
    masks = [_masks_np(boundary=True), _masks_np(boundary=False)]

    in_maps = []
    for c in range(NCORES):
        b, qd = divmod(c, 4)
        lo = qd * SQ - W
        hi = qd * SQ + SQ
        vs = val[max(lo, 0):hi, b, :]
        if lo < 0:
            vs = np.concatenate([np.zeros((-lo, D), np.float32), vs], axis=0)
        in_maps.append({
            "valT": np.ascontiguousarray(vs.T.astype(bf)),
            "wq": wq_s, "wk": wk_c, "wv": wv_c, "bq": bq_s,
            "masks": masks[0 if qd == 0 else 1],
        })

    nc = _get_nc()
    trace = os.environ.get("BASS_KERNEL_TRACE", "0") == "1"
    kwargs = {}
    if trace:
        _install_ntff_hook()
        kwargs = dict(trace=True, tmpdir=os.environ.get("BASS_KERNEL_TRACE_DIR") or None)
    res = run_bass_kernel_spmd(nc, in_maps, list(range(NCORES)), **kwargs)
    _CACHE["last_result"] = res

    out = np.empty((S, B, D), np.float32)
    for c in range(NCORES):
        b, qd = divmod(c, 4)
        raw = res.results[c]["out"].astype(np.float32).reshape(SQ, H, VA)
        out[qd * SQ:(qd + 1) * SQ, b, :] = (
            raw[:, :, 0:HD] / raw[:, :, HD:HD + 1]).reshape(SQ, D)
    out += bv
    return out


# revision 31
# speedup vs baseline: 1.0055x; 1.0055x over previous
"""Trainium2 Bass kernel for LocalSelfAttention (sliding-window, causal).

Problem: val (S=4096, B=2, D=768); q/k/v projections then Longformer-style
banded causal attention, window = 256 lookback (keys j in [i-256, i]).

Sharding: 8 cores = batch (2) x sequence quarters (4). Each core handles
1024 queries of one batch element and receives a 256-row key/value halo
(recomputed locally from val rows; no inter-core communication).

Math simplifications (exact up to float rounding):
  - bk dropped: per-query additive constant q.bk cancels in softmax.
  - bv added on host at the end: sum_j p_j (v0_j + bv) = (PV)/Z + bv.
  - no max-subtraction in softmax: scores ~ N(0,1), |s| < ~8, exp is safe.
  - 1/sqrt(hd) folded into Wq/bq on host.

v3 layout (all bf16 on the PE):
  - phase A: all q projections (DMA-paced ramp; ACT does the bias adds).
  - m-loop: k-proj(m) chunks, scores for head pair j=m, and v-proj chunks
    interleaved at ~1us granularity so PE / ACT(exp) / DVE(copies+masks)
    all stay fed.
  - scores per (head, kvt 128-key tile): one matmul keys x up-to-384
    contiguous queries (the exact span attending that key tile). Mid
    psum tiles hold a kvt pair at col 0 / 512 (bank-aligned); one wide
    exp covers both blocks. Banded validity = triangle-strip multiplies
    on DVE; per-core combined mask handles the first two (halo) tiles.
  - PV: probs-stationary matmuls accumulate all 12 heads into one
    bank-aligned psum tile per 128-query tile; ones-column in v gives
    softmax denominators; division on host from bf16 outputs.
"""

import os
import numpy as np
import ml_dtypes

S, B, D = 4096, 2, 768
H, HD = 12, 64
W = 256
NCORES = 8
SQ = S // 4            # 1024 queries per core
SKV = SQ + W           # 1280 kv rows (halo)
NQT = SQ // 128        # 8 query tiles
ND = D // 128          # 6 feature tiles (= head pairs)
NKVT = SKV // 128      # 10 kv tiles
VA = HD + 2            # 66: per-head v width incl. ones column + pad
SCALE = 1.0 / np.sqrt(HD).astype(np.float32)  # 0.125

_CACHE = {}


def _qlo(kvt):
    return max(0, (kvt - 2) * 128)


def _qhi(kvt):
    return min(SQ, (kvt + 1) * 128)


def _masks_np(boundary: bool) -> np.ndarray:
    """(128, 896) bf16: [M0 (384) | A (128) | B (128) | A (128) | pad (128)].

    Partition p = key-within-tile, free c = query column within block.
    A[p,c] = 1 iff c >= p   (left strip of interior blocks)
    B[p,c] = 1 iff c <= p   (right strip: col c-256 valid iff c-256 <= p)
    M0 covers the t2=0 tile [kvt0 block (128) | kvt1 block (256)]:
      interior: [B | ones | B]; boundary (seq start): zeros (halo keys < 0).
    [A|B] at 384 serves the interior strip pairs (stride-128 view); the
    [A.. A] pair at 384/640 serves t2=4 via a stride-256 view.
    """
    p = np.arange(128)[:, None]
    c = np.arange(128)[None, :]
    A = (c >= p).astype(np.float32)
    Bm = (c <= p).astype(np.float32)
    ones = np.ones((128, 128), np.float32)
    zero = np.zeros((128, 128), np.float32)
    if boundary:
        M0 = np.zeros((128, 384), np.float32)
    else:
        M0 = np.concatenate([Bm, ones, Bm], axis=1)
    m = np.concatenate([M0, A, Bm, A, zero], axis=1)
    return np.ascontiguousarray(m.astype(ml_dtypes.bfloat16))


# scores psum/probs tile layout per t2 (pair of kv tiles):
# t2=0: kvt0 at [0:128), kvt1 at [128:384), width 512 (psum) / 384 (probs)
# t2=1..3: kvt=2t2 at [0:384), kvt=2t2+1 at [512:896); psum width 1024
#          (bank aligned), probs width 896; [384:512) is junk (one wide exp)
# t2=4: kvt8 at [0:256), kvt9 at [256:384), width 512 / 384
def _ps_width(t2):
    return 512 if t2 in (0, 4) else 1024


def _pr_width(t2):
    return 512 if t2 in (0, 4) else 1024


def _blk_off(kvt):
    t2, kk = divmod(kvt, 2)
    if t2 == 0:
        return 0 if kk == 0 else 128
    if t2 == 4:
        return 0 if kk == 0 else 256
    return 0 if kk == 0 else 512


# PV psum col offset per head (66 fp32 cols; no matmul crosses a 2KB bank)
def _pv_off(h):
    return h * 66 if h < 7 else 512 + (h - 7) * 66


def _build_nc():
    import concourse.bacc as bacc
    import concourse.mybir as mybir
    from concourse.tile import TileContext

    f32 = mybir.dt.float32
    bf16 = mybir.dt.bfloat16
    AF = mybir.ActivationFunctionType

    nc = bacc.Bacc(trn_type="TRN2", debug=False, num_devices=NCORES)

    valT_d = nc.dram_tensor("valT", [D, SKV], bf16, kind="ExternalInput").ap()
    wq_d = nc.dram_tensor("wq", [D, D], bf16, kind="ExternalInput").ap()
    wk_d = nc.dram_tensor("wk", [D, D], bf16, kind="ExternalInput").ap()
    wv_d = nc.dram_tensor("wv", [D, D], bf16, kind="ExternalInput").ap()
    bq_d = nc.dram_tensor("bq", [128, ND], f32, kind="ExternalInput").ap()
    masks_d = nc.dram_tensor("masks", [128, 896], bf16, kind="ExternalInput").ap()
    out_d = nc.dram_tensor("out", [SQ, H * VA], bf16, kind="ExternalOutput").ap()

    with TileContext(nc) as tc:
        with tc.tile_pool(name="persist", bufs=1) as pp:
            qT = [pp.tile([128, SQ], bf16, name=f"qT{m}", tag=f"qT{m}") for m in range(ND)]
            kT = [pp.tile([128, SKV], bf16, name=f"kT{m}", tag=f"kT{m}") for m in range(ND)]
            vaug = [pp.tile([128, H * VA], bf16, name=f"vaug{t}", tag=f"vaug{t}") for t in range(NKVT)]
            bqt = pp.tile([128, ND], f32, name="bqt", tag="bqt")
            maskt = pp.tile([128, 896], bf16, name="maskt", tag="maskt")
            probs = [[pp.tile([128, _pr_width(t2)], bf16, name=f"pr{h}_{t2}", tag=f"pr{h}_{t2}")
                      for t2 in range(5)] for h in range(H)]

            valT_t, wq_t, wk_t, wv_t = [], [], [], []
            with tc.tile_pool(name="weights", bufs=1) as wp:
                # --- input DMAs. Per-queue transfers run strictly in issue
                # order, so priority is enforced within a queue: sync carries
                # valT (needed first) then wk then masks then wv; scalar
                # carries bq+wq (light, frees scalar for ACT work quickly).
                # per-queue transfers run strictly in issue order: sync
                # carries valT (needed first) then wk/masks/wv; scalar carries
                # bq+wq (light; scalar engine is free of DMA work afterwards)
                nc.scalar.dma_start(bqt[:], bq_d)
                for k in range(ND):
                    tw = wp.tile([128, D], bf16, name=f"wqt{k}", tag=f"wqt{k}")
                    nc.scalar.dma_start(tw[:], wq_d[k * 128:(k + 1) * 128, :])
                    wq_t.append(tw)
                for k in range(ND):
                    tv = wp.tile([128, SKV], bf16, name=f"valTt{k}", tag=f"valTt{k}")
                    nc.sync.dma_start(tv[:], valT_d[k * 128:(k + 1) * 128, :])
                    valT_t.append(tv)
                for k in range(ND):
                    t = wp.tile([128, D], bf16, name=f"wkt{k}", tag=f"wkt{k}")
                    nc.sync.dma_start(t[:], wk_d[k * 128:(k + 1) * 128, :])
                    wk_t.append(t)
                nc.sync.dma_start(maskt[:], masks_d)
                for k in range(ND):
                    t = wp.tile([128, D], bf16, name=f"wvt{k}", tag=f"wvt{k}")
                    nc.sync.dma_start(t[:], wv_d[k * 128:(k + 1) * 128, :])
                    wv_t.append(t)

                for t in range(NKVT):
                    va3 = vaug[t][:].rearrange("p (h c) -> p h c", c=VA)
                    nc.vector.memset(va3[:, :, HD:HD + 1], 1.0)
                    nc.vector.memset(va3[:, :, HD + 1:VA], 0.0)

                vchunks = [(t, lo, hi, h0) for t in range(NKVT)
                           for lo, hi, h0 in ((0, 512, 0), (512, 768, 8))]
                vidx = 0

                # ---- phase A: all q projections. Six accumulation chains
                # interleaved in 3-matmul half-chains so all chains advance as
                # valT tiles land (DMA-paced ramp), without per-MM bank cycling.
                with tc.tile_pool(name="phA", bufs=1, space="PSUM") as phA:
                    for grp in range(2):
                        chains = [(m, ch) for m in range(3 * grp, 3 * grp + 3)
                                  for ch in range(2)]
                        tiles = [phA.tile([128, 512], f32, name=f"psq{i}", tag=f"psq{i}")
                                 for i in range(6)]
                        for half in range(2):
                            for i, (m, ch) in enumerate(chains):
                                for k in range(3 * half, 3 * half + 3):
                                    nc.tensor.matmul(
                                        tiles[i][:],
                                        wq_t[k][:, m * 128:(m + 1) * 128],
                                        valT_t[k][:, W + ch * 512:W + (ch + 1) * 512],
                                        start=(k == 0), stop=(k == ND - 1))
                        for i, (m, ch) in enumerate(chains):
                            nc.vector.tensor_scalar_add(
                                qT[m][:, ch * 512:(ch + 1) * 512], tiles[i][:],
                                bqt[:, m:m + 1])

                with tc.tile_pool(name="pps", bufs=2, space="PSUM") as pps, \
                     tc.tile_pool(name="scps", bufs=3, space="PSUM") as scp:

                    def v_chunk():
                        nonlocal vidx
                        if vidx >= len(vchunks):
                            return
                        t, lo, hi, h0 = vchunks[vidx]
                        vidx += 1
                        ps = pps.tile([128, hi - lo], f32, name="psv", tag="psproj")
                        for k in range(ND):
                            nc.tensor.matmul(
                                ps[:],
                                valT_t[k][:, t * 128:(t + 1) * 128],
                                wv_t[k][:, lo:hi],
                                start=(k == 0), stop=(k == ND - 1))
                        nh = (hi - lo) // HD
                        src = ps[:].rearrange("p (h c) -> p h c", c=HD)
                        dst = vaug[t][:].rearrange(
                            "p (h c) -> p h c", c=VA)[:, h0:h0 + nh, 0:HD]
                        nc.vector.tensor_copy(dst, src)

                    def scores_t2(m, t2):
                        ps_s = [scp.tile([128, _ps_width(t2)], f32,
                                         name=f"pss{hh}", tag="scores")
                                for hh in range(2)]
                        for kk in range(2):
                            kvt = 2 * t2 + kk
                            qlo, qhi = _qlo(kvt), _qhi(kvt)
                            for hh in range(2):
                                nc.tensor.matmul(
                                    ps_s[hh][:, _blk_off(kvt):_blk_off(kvt) + qhi - qlo],
                                    kT[m][hh * 64:hh * 64 + 64, kvt * 128:(kvt + 1) * 128],
                                    qT[m][hh * 64:hh * 64 + 64, qlo:qhi],
                                    start=True, stop=True,
                                    tile_position=(hh * 64, 0))
                        for hh in range(2):
                            h = 2 * m + hh
                            P = probs[h][t2]
                            if t2 in (0, 4):
                                nc.scalar.activation(P[:, 0:384], ps_s[hh][:, 0:384], AF.Exp)
                            else:
                                nc.scalar.activation(P[:, 0:896], ps_s[hh][:, 0:896], AF.Exp)
                            # banded validity masks (strip pairs as one
                            # strided op: cols {off, off+256}, 128 wide)
                            if t2 == 0:
                                nc.vector.tensor_mul(
                                    P[:, 0:384], P[:, 0:384], maskt[:, 0:384])
                            elif t2 == 4:
                                Pv = P[:, 0:512].rearrange(
                                    "p (s c) -> p s c", c=256)[:, :, 0:128]
                                Mv = maskt[:, 384:896].rearrange(
                                    "p (s c) -> p s c", c=256)[:, :, 0:128]
                                nc.vector.tensor_mul(Pv, Pv, Mv)
                            else:
                                Mv = maskt[:, 384:640].rearrange(
                                    "p (s c) -> p s c", c=128)
                                for off, eng in ((0, nc.gpsimd), (512, nc.vector)):
                                    Pv = P[:, off:off + 512].rearrange(
                                        "p (s c) -> p s c", c=256)[:, :, 0:128]
                                    eng.tensor_mul(Pv, Pv, Mv)

                    # ---- m-loop: k-proj(m) + scores(j=m) + v chunks ----
                    for m in range(ND):
                        nv = (3, 3, 3, 3, 4, 4)[m]
                        for lo, hi in ((0, 512), (512, 1024), (1024, 1280)):
                            ps = pps.tile([128, hi - lo], f32, name="psk", tag="psproj")
                            for k in range(ND):
                                nc.tensor.matmul(
                                    ps[:],
                                    wk_t[k][:, m * 128:(m + 1) * 128],
                                    valT_t[k][:, lo:hi],
                                    start=(k == 0), stop=(k == ND - 1))
                            nc.vector.tensor_copy(kT[m][:, lo:hi], ps[:])
                            if lo == 0:
                                for _ in range((nv + 2) // 3):
                                    v_chunk()
                                scores_t2(m, 0)
                                scores_t2(m, 1)
                            elif lo == 512:
                                for _ in range((nv + 1) // 3):
                                    v_chunk()
                                scores_t2(m, 2)
                                scores_t2(m, 3)
                            else:
                                for _ in range(nv // 3):
                                    v_chunk()
                                scores_t2(m, 4)
                    while vidx < len(vchunks):
                        v_chunk()

                # ---- PV phase ----
                with tc.tile_pool(name="pvps", bufs=3, space="PSUM") as pvp, \
                     tc.tile_pool(name="outp", bufs=2) as op:
                    for qt in range(NQT):
                        ps_o = pvp.tile([128, 1024], f32, name="pso", tag="pv")
                        for h in range(H):
                            for n, kvt in enumerate((qt, qt + 1, qt + 2)):
                                t2 = kvt // 2
                                coff = _blk_off(kvt) + qt * 128 - _qlo(kvt)
                                nc.tensor.matmul(
                                    ps_o[:, _pv_off(h):_pv_off(h) + VA],
                                    probs[h][t2][:, coff:coff + 128],
                                    vaug[kvt][:, h * VA:(h + 1) * VA],
                                    start=(n == 0), stop=(n == 2))
                        osb = op.tile([128, H * VA], bf16, name="osb", tag="osb")
                        nc.vector.tensor_copy(osb[:, 0:462], ps_o[:, 0:462])
                        nc.vector.tensor_copy(osb[:, 462:792], ps_o[:, 512:842])
                        nc.sync.dma_start(out_d[qt * 128:(qt + 1) * 128, :], osb[:])
    nc.compile()
    return nc


def _get_nc():
    if "nc" not in _CACHE:
        _CACHE["nc"] = _build_nc()
    return _CACHE["nc"]


def _install_ntff_hook():
    """Provide antenv.axon_hooks (absent in this image) so bass_utils can
    NTFF-profile under axon, using trn_agent_boot's ctypes hook builder."""
    import sys
    import types
    try:
        from antenv.axon_hooks import get_axon_ntff_profile_hook  # noqa: F401
        return
    except ImportError:
        pass
    try:
        import antenv
        from trn_agent_boot.trn_boot import _ntff_profile_via_ctypes
        hook = _ntff_profile_via_ctypes("/opt/axon/libaxon_pjrt.so")
        mod = types.ModuleType("antenv.axon_hooks")
        mod.get_axon_ntff_profile_hook = lambda: hook
        mod.set_axon_ntff_profile_hook = lambda h: None
        sys.modules["antenv.axon_hooks"] = mod
        antenv.axon_hooks = mod
    except Exception as e:  # profiling is best-effort
        print(f"ntff hook install failed: {e}")


def kernel(val, Wq, bq, Wk, bk, Wv, bv):
    from concourse.bass_utils import run_bass_kernel_spmd

    val = np.asarray(val, dtype=np.float32)
    Wq = np.asarray(Wq, dtype=np.float32)
    bq = np.asarray(bq, dtype=np.float32)
    Wk = np.asarray(Wk, dtype=np.float32)
    Wv = np.asarray(Wv, dtype=np.float32)
    bv = np.asarray(bv, dtype=np.float32)

    bf = ml_dtypes.bfloat16
    wq_s = np.ascontiguousarray((Wq * SCALE).astype(bf))
    bq_s = np.ascontiguousarray((bq * SCALE).reshape(ND, 128).T)
    wk_c = np.ascontiguousarray(Wk.astype(bf))
    wv_c = np.ascontiguousarray(Wv.astype(bf))
    masks = [_masks_np(boundary=True), _masks_np(boundary=False)]

    in_maps = []
    for c in range(NCORES):
        b, qd = divmod(c, 4)
        lo = qd * SQ - W
        hi = qd * SQ + SQ
        vs = val[max(lo, 0):hi, b, :]
        if lo < 0:
            vs = np.concatenate([np.zeros((-lo, D), np.float32), vs], axis=0)
        in_maps.append({
            "valT": np.ascontiguousarray(vs.T.astype(bf)),
            "wq": wq_s, "wk": wk_c, "wv": wv_c, "bq": bq_s,
            "masks": masks[0 if qd == 0 else 1],
        })

    nc = _get_nc()
    trace = os.environ.get("BASS_KERNEL_TRACE", "0") == "1"
    kwargs = {}
    if trace:
        _install_ntff_hook()
        kwargs = dict(trace=True, tmpdir=os.environ.get("BASS_KERNEL_TRACE_DIR") or None)
    res = run_bass_kernel_spmd(nc, in_maps, list(range(NCORES)), **kwargs)
    _CACHE["last_result"] = res

    out = np.empty((S, B, D), np.float32)
    for c in range(NCORES):
        b, qd = divmod(c, 4)
        raw = res.results[c]["out"].astype(np.float32).reshape(SQ, H, VA)
        out[qd * SQ:(qd + 1) * SQ, b, :] = (
            raw[:, :, 0:HD] / raw[:, :, HD:HD + 1]).reshape(SQ, D)
    out += bv
    return out
